# revision 56
# baseline (speedup 1.0000x reference)
"""Self-contained 8-core Trainium2 Bass kernel for MultiHeadAttention.

Problem: B=2, S=2048, D=1024, H=16 heads (hd=64), f32, self-attention
(no mask), eval mode (dropout = identity).

Sharding: data-parallel over B (2) x tensor-parallel over heads (4 groups
of 4 heads) = 8 cores. Each core computes, for its batch b and its 4
heads: Q/K/V projections (column-sliced), attention, and a partial
output projection (row-sliced Wo). Host sums the 4 partials per batch
and adds the (bv @ Wo + bo) correction (bv never enters the kernel:
ctx rows sum probs to 1, so (ctx+bv) @ Wo = ctx @ Wo + bv @ Wo).

v2 design (vs v1 f32r baseline at ~322us):
  - All matmul operands bf16 (host-cast): FWL weight loads, half DMA,
    half DVE copy cost. PSUM accumulation stays f32, output f32.
  - Scores per head pair computed as two concurrent row-tiled K=64
    matmuls (tile_position=(64h,0)) -- halves scores PE time.
  - reciprocal_approx_fast (custom DVE, ~51 ULP) replaces the 3.3us
    bit-exact reciprocal in the softmax-denominator normalization.
  - One interleaved schedule: attention(pair0) starts as soon as the
    first K-chunk projections land (xt DMA'd qc-major); V projection and
    pair-1 K/Q projections and the output projection are spread into the
    ACT(exp)-bound attention windows so the PE never idles.
  - PSUM budget (8 banks): scores tag 2 bufs x 2 banks; ctx tag 3 bufs
    x 1 bank; aux (late proj + outproj) 1 buf x 1 bank.

Algebraic simplifications (exact, from v1):
  - bk dropped: softmax over k is invariant to the per-q constant Q.bk.
  - softmax without max subtraction (scores bounded, exp safe in f32).
  - bq folded into Q^T as a per-partition bias.
  - row normalization deferred past the P@V matmul (scale ctx instead
    of probs); row sums obtained free via an appended ones-column in V.
"""

import sys

sys.path.insert(0, "/opt/trn_rl_repo")

import numpy as np

B, S, D, H, HD = 2, 2048, 1024, 16, 64
HPC = 4  # heads per core
NCORES = 8
DC = D // 128  # 8 contraction chunks
ST = S // 128  # 16 s-tiles
QCW = 512  # q chunk width
QC = S // QCW  # 4 q chunks
KT = S // 128  # 16 k tiles

_CACHE = {}
APPROX_RECIP = True
DIRECT_OUT_DMA = False


def _build(repeat=1):
    import concourse.bass as bass  # noqa: F401
    import concourse.mybir as mybir
    import concourse.tile as tile
    from concourse import bacc

    F32 = mybir.dt.float32
    BF16 = mybir.dt.bfloat16
    AF = mybir.ActivationFunctionType

    nc = bacc.Bacc("TRN2", target_bir_lowering=False, debug=False)

    xt_d = nc.dram_tensor("xt", [D, S], BF16, kind="ExternalInput")
    wq_d = nc.dram_tensor("wq", [D, HPC * HD], BF16, kind="ExternalInput")
    wk_d = nc.dram_tensor("wk", [D, HPC * HD], BF16, kind="ExternalInput")
    wv_d = nc.dram_tensor("wv", [D, HPC * HD], BF16, kind="ExternalInput")
    wo_d = nc.dram_tensor("wo", [HPC * HD, D], BF16, kind="ExternalInput")
    bq_d = nc.dram_tensor("bq2", [128, 2], F32, kind="ExternalInput")
    out_d = nc.dram_tensor("out_p", [S, D], F32, kind="ExternalOutput")

    with tile.TileContext(nc) as tc:
        with (
            tc.tile_pool(name="wp", bufs=1) as wp,
            tc.tile_pool(name="xp", bufs=1) as xp,
            tc.tile_pool(name="qk", bufs=1) as qk,
            tc.tile_pool(name="vp", bufs=1) as vp,
            tc.tile_pool(name="ep", bufs=3) as ep,
            tc.tile_pool(name="cp", bufs=1) as cp,
            tc.tile_pool(name="mp", bufs=2) as mp,
            tc.tile_pool(name="op", bufs=2) as op,
            tc.tile_pool(name="pp", bufs=2, space="PSUM") as pp,
        ):
            # ---- one sync-queue DMA stream, ordered by first consumption:
            # wk/wq gate the first projections; xt is qc-major so pair-0
            # attention starts as soon as the first K-chunks land.
            wk_t = wp.tile([128, DC, HPC * HD], BF16, tag="wk")
            nc.sync.dma_start(wk_t[:], wk_d.rearrange("(c p) n -> p c n", p=128))
            xt_t = xp.tile([128, DC, S], BF16, tag="xt")
            xt_r = xt_d.rearrange("(c p) s -> p c s", p=128)

            def dma_xt(qc, half=None):
                qs = slice(qc * QCW, (qc + 1) * QCW)
                cs = slice(0, DC) if half is None else slice(half * DC // 2, (half + 1) * DC // 2)
                nc.sync.dma_start(xt_t[:, cs, qs], xt_r[:, cs, qs])

            dma_xt(0, 0)
            dma_xt(0, 1)
            wq_t = wp.tile([128, DC, HPC * HD], BF16, tag="wq")
            nc.sync.dma_start(wq_t[:], wq_d.rearrange("(c p) n -> p c n", p=128))
            bq_t = wp.tile([128, 2], F32, tag="bq")
            nc.sync.dma_start(bq_t[:], bq_d[:])
            wv_t = wp.tile([128, DC, HPC * HD], BF16, tag="wv")
            nc.sync.dma_start(wv_t[:], wv_d.rearrange("(c p) n -> p c n", p=128))
            dma_xt(1)
            wo_t = wp.tile([128, 2, D], BF16, tag="wo")
            nc.sync.dma_start(wo_t[:], wo_d.rearrange("(c p) n -> p c n", p=128))
            dma_xt(2)
            dma_xt(3)



            import contextlib
            if repeat > 1:
                _engs = [mybir.EngineType.PE, mybir.EngineType.Activation,
                         mybir.EngineType.DVE, mybir.EngineType.SP,
                         mybir.EngineType.Pool]
                rep_ctx = tc.For_i(0, repeat, hint_engines=_engs, staggered_reset=True)
            else:
                rep_ctx = contextlib.nullcontext()
            with rep_ctx:
                # ---- persistent SBUF tiles
                # v1 head block layout: [64 ones | 64 v-dims] so the PV matmul
                # replicates the softmax denominator onto PSUM partitions 0-63
                # (base-0 for the custom-DVE reciprocal) and ctx onto 64-127.
                v1_t = vp.tile([128, ST, HPC * 128], BF16, tag="v1")
                qt_tiles = [qk.tile([128, S], BF16, tag=f"qt{p}", name=f"qt{p}") for p in range(2)]
                kt_tiles = [qk.tile([128, S], BF16, tag=f"kt{p}", name=f"kt{p}") for p in range(2)]
                ctxt_tiles = [cp.tile([128, S], BF16, tag=f"ct{p}", name=f"ct{p}") for p in range(2)]

                def v_proj(st, tag="big"):
                    vps = pp.tile([128, HPC * HD], F32, tag=tag, name="vps",
                                  bufs=1 if tag == "aux" else 2)
                    for c in range(DC):
                        nc.tensor.matmul(
                            vps[:],
                            xt_t[:, c, st * 128:(st + 1) * 128],
                            wv_t[:, c, :],
                            start=(c == 0),
                            stop=(c == DC - 1),
                        )
                    with nc.allow_low_precision(reason="bf16 matmul operands"):
                        nc.vector.tensor_copy(
                            v1_t[:, st, :].rearrange("p (h c) -> p h c", c=128)[:, :, 64:128],
                            vps[:].rearrange("p (h c) -> p h c", c=64),
                        )

                def kt_proj(pair, qc, tag="big"):
                    qs = slice(qc * QCW, (qc + 1) * QCW)
                    kps = pp.tile([128, QCW], F32, tag=tag, name="kps",
                                  bufs=1 if tag == "aux" else 2)
                    for c in range(DC):
                        nc.tensor.matmul(
                            kps[:],
                            wk_t[:, c, pair * 128:(pair + 1) * 128],
                            xt_t[:, c, qs],
                            start=(c == 0),
                            stop=(c == DC - 1),
                        )
                    with nc.allow_low_precision(reason="bf16 matmul operands"):
                        nc.vector.tensor_copy(kt_tiles[pair][:, qs], kps[:])

                def qt_proj(pair, qc, tag="big"):
                    qs = slice(qc * QCW, (qc + 1) * QCW)
                    qps = pp.tile([128, QCW], F32, tag=tag, name="qps",
                                  bufs=1 if tag == "aux" else 2)
                    for c in range(DC):
                        nc.tensor.matmul(
                            qps[:],
                            wq_t[:, c, pair * 128:(pair + 1) * 128],
                            xt_t[:, c, qs],
                            start=(c == 0),
                            stop=(c == DC - 1),
                        )
                    with nc.allow_low_precision(reason="bf16 score operands"):
                        nc.vector.tensor_scalar_add(
                            qt_tiles[pair][:, qs], qps[:], bq_t[:, pair:pair + 1]
                        )

                # ---- software-pipelined attention: PV trails scores by 8
                # r-slots via staged expt tiles, so the PE never waits on the
                # ACT exp stream and the ACT engine is fed continuously.
                SEQ = [(p, q) for p in range(2) for q in range(QC)]
                expt_store = {}
                ctx_store = {}

                def sc_emit(w, r):
                    pair, qc = SEQ[w]
                    qs = slice(qc * QCW, (qc + 1) * QCW)
                    sreg = pp.tile([128, 2 * QCW], F32, tag="big")
                    expt = ep.tile([128, 2 * QCW], BF16, tag="exp", bufs=20)
                    for h in range(2):
                        nc.tensor.matmul(
                            sreg[:, h * QCW:(h + 1) * QCW],
                            kt_tiles[pair][64 * h:64 * (h + 1), r * 128:(r + 1) * 128],
                            qt_tiles[pair][64 * h:64 * (h + 1), qs],
                            start=True,
                            stop=True,
                            tile_position=(64 * h, 0),
                        )
                    nc.scalar.activation(expt[:], sreg[:], AF.Exp, scale=0.125)
                    expt_store[(w, r)] = expt

                def pv_emit(w, r):
                    pair, qc = SEQ[w]
                    if r == 0:
                        ctx_store[w] = [pp.tile([128, QCW], F32, tag="ctx", name=f"ctx{_h}", bufs=3) for _h in range(2)]
                    ctx_ps = ctx_store[w]
                    expt = expt_store.pop((w, r))
                    for h in range(2):
                        hh = 2 * pair + h
                        nc.tensor.matmul(
                            ctx_ps[h][:],
                            v1_t[:, r, 128 * hh:128 * hh + 128],
                            expt[:, h * QCW:(h + 1) * QCW],
                            start=(r == 0),
                            stop=(r == KT - 1),
                        )

                def norm_emit(w, norm_chunks=1):
                    pair, qc = SEQ[w]
                    qs = slice(qc * QCW, (qc + 1) * QCW)
                    ctx_ps = ctx_store.pop(w)
                    cw = QCW // norm_chunks
                    for cc in range(norm_chunks):
                        pcs = slice(cc * cw, (cc + 1) * cw)
                        for h in range(2):
                            bct = mp.tile([64, cw], F32, tag="bc")
                            if APPROX_RECIP:
                                nc.vector.reciprocal_approx_fast(bct[:], ctx_ps[h][0:64, pcs])
                            else:
                                nc.vector.reciprocal(bct[:], ctx_ps[h][0:64, pcs])
                            with nc.allow_low_precision(reason="bf16 matmul operands"):
                                nc.vector.tensor_mul(
                                    ctxt_tiles[pair][64 * h:64 * (h + 1), qs][:, pcs],
                                    ctx_ps[h][64:128, pcs],
                                    bct[:],
                                )

                def outproj(qc, subs=(0, 1, 2, 3), tail=False, psum_tag="aux"):
                    # tail=True: copies go to the scalar engine, which is idle
                    # once the exp stream has drained (keeps the in-order PE
                    # queue from stalling on DVE copy backpressure).
                    for sub in subs:
                        q0 = qc * QCW + sub * 128
                        for d2 in range(2):
                            ops = pp.tile([128, 512], F32, tag=psum_tag,
                                          bufs=1 if psum_tag == "aux" else 2, name="ops")
                            for pair in range(2):
                                nc.tensor.matmul(
                                    ops[:],
                                    ctxt_tiles[pair][:, q0:q0 + 128],
                                    wo_t[:, pair, d2 * 512:(d2 + 1) * 512],
                                    start=(pair == 0),
                                    stop=(pair == 1),
                                )
                            osb = op.tile([128, 512], F32, tag="osb")
                            if tail:
                                nc.scalar.copy(osb[:], ops[:])
                            else:
                                nc.vector.tensor_copy(osb[:], ops[:])
                            nc.sync.dma_start(out_d[q0:q0 + 128, d2 * 512:(d2 + 1) * 512], osb[:])

                # ================= schedule =================
                # Window w runs scores/exp for SEQ[w] while PV for the trailing
                # half of SEQ[w-1] and the leading half of SEQ[w] accumulate
                # from staged expt tiles. Projections/outproj fill PE slack.
                # head: run the first K/Q projections with interleaved
                # c-chunks so both trail the xt DMA tightly.
                kps0 = pp.tile([128, QCW], F32, tag="big", bufs=2, name="kps0")
                qps0 = pp.tile([128, QCW], F32, tag="big", bufs=2, name="qps0")
                for c in range(DC):
                    nc.tensor.matmul(kps0[:], wk_t[:, c, 0:128], xt_t[:, c, 0:QCW],
                                     start=(c == 0), stop=(c == DC - 1))
                    nc.tensor.matmul(qps0[:], wq_t[:, c, 0:128], xt_t[:, c, 0:QCW],
                                     start=(c == 0), stop=(c == DC - 1))
                with nc.allow_low_precision(reason="bf16 matmul operands"):
                    nc.vector.tensor_copy(kt_tiles[0][:, 0:QCW], kps0[:])
                    nc.vector.tensor_scalar_add(
                        qt_tiles[0][:, 0:QCW], qps0[:], bq_t[:, 0:1])
                nc.vector.memset(
                    v1_t[:].rearrange("p s (h c) -> p s h c", c=128)[:, :, :, 0:64],
                    1.0,
                )

                def hooks(w, r):
                    if w == 0:
                        if r == 2:
                            qt_proj(0, 1, tag="aux")
                        elif r in (4, 8, 12):
                            kt_proj(0, r // 4)
                        if r >= 8:
                            v_proj(r - 8)
                    elif w == 1:
                        if r < 8:
                            v_proj(r + 8)
                        if r == 2:
                            qt_proj(0, 2, tag="aux")
                    elif w == 2:
                        if r == 2:
                            qt_proj(0, 3, tag="aux")
                        elif r == 6:
                            kt_proj(1, 0, tag="aux")
                        elif r == 11:
                            kt_proj(1, 1, tag="aux")
                    elif w == 3:
                        if r == 2:
                            qt_proj(1, 0, tag="aux")
                        elif r == 6:
                            kt_proj(1, 2, tag="aux")
                        elif r == 11:
                            kt_proj(1, 3, tag="aux")
                    elif w == 4:
                        if r == 2:
                            qt_proj(1, 1, tag="aux")
                    elif w == 5:
                        if r == 2:
                            qt_proj(1, 2, tag="aux")
                        elif r == 9:
                            outproj(0, subs=(0, 1))
                        elif r == 12:
                            outproj(0, subs=(2, 3))
                    elif w == 6:
                        if r == 2:
                            qt_proj(1, 3, tag="aux")
                        elif r == 9:
                            outproj(1, subs=(0, 1))
                        elif r == 12:
                            outproj(1, subs=(2, 3))
                    elif w == 7:
                        if r in (9, 11, 13, 15):
                            outproj(2, subs=((r - 9) // 2,), tail=True)

                NW = len(SEQ)
                for w in range(NW):
                    for r in range(KT):
                        hooks(w, r)
                        sc_emit(w, r)
                        if r < 8:
                            if w > 0:
                                pv_emit(w - 1, r + 8)
                                if r == 7:
                                    norm_emit(w - 1)
                        else:
                            pv_emit(w, r - 8)
                # tail: drain the last window's PV, norm in chunks, project out
                for r in range(8, KT):
                    pv_emit(NW - 1, r)
                norm_emit(NW - 1, norm_chunks=4)
                outproj(QC - 1, tail=True, psum_tag="big")

    nc.compile()
    return nc


def _get_nc(repeat=1):
    key = (repeat, 3, APPROX_RECIP, DIRECT_OUT_DMA)
    if key not in _CACHE:
        _CACHE[key] = _build(repeat)
    return _CACHE[key]


def _make_in_maps(query_input, Wq, bq, Wk, Wv, Wo):
    from ml_dtypes import bfloat16

    x = np.asarray(query_input, dtype=np.float32)
    in_maps = []
    for core in range(NCORES):
        b, g = divmod(core, NCORES // B)
        cs = slice(g * HPC * HD, (g + 1) * HPC * HD)
        in_maps.append({
            "xt": np.ascontiguousarray(x[b].T).astype(bfloat16),
            "wq": np.ascontiguousarray(Wq[:, cs]).astype(bfloat16),
            "wk": np.ascontiguousarray(Wk[:, cs]).astype(bfloat16),
            "wv": np.ascontiguousarray(Wv[:, cs]).astype(bfloat16),
            "wo": np.ascontiguousarray(Wo[cs, :]).astype(bfloat16),
            "bq2": np.ascontiguousarray(bq[cs].reshape(2, 128).T),
        })
    return in_maps


def kernel(query_input, Wq, bq, Wk, bk, Wv, bv, Wo, bo):
    from concourse.bass_utils import run_bass_kernel_spmd

    Wq = np.asarray(Wq, np.float32)
    Wk = np.asarray(Wk, np.float32)
    Wv = np.asarray(Wv, np.float32)
    Wo = np.asarray(Wo, np.float32)
    bq = np.asarray(bq, np.float32)
    bv = np.asarray(bv, np.float32)
    bo = np.asarray(bo, np.float32)

    nc = _get_nc()
    in_maps = _make_in_maps(query_input, Wq, bq, Wk, Wv, Wo)
    res = run_bass_kernel_spmd(nc, in_maps, core_ids=list(range(NCORES)))

    gpc = NCORES // B  # groups per batch
    out = np.zeros((B, S, D), np.float32)
    for core in range(NCORES):
        b = core // gpc
        out[b] += res.results[core]["out_p"]
    # bv correction (exact) + bo, applied once on the full output
    out += (bv @ Wo + bo)[None, None, :]
    return out


# revision 57
# speedup vs baseline: 1.1624x; 1.1624x over previous
"""Self-contained 8-core Trainium2 Bass kernel for MultiHeadAttention.

Problem: B=2, S=2048, D=1024, H=16 heads (hd=64), f32, self-attention
(no mask), eval mode (dropout = identity).

Sharding: data-parallel over B (2) x tensor-parallel over heads (4 groups
of 4 heads) = 8 cores. Each core computes, for its batch b and its 4
heads: Q/K/V projections (column-sliced), attention, and a partial
output projection (row-sliced Wo). Host sums the 4 partials per batch
and adds the (bv @ Wo + bo) correction (bv never enters the kernel:
ctx rows sum probs to 1, so (ctx+bv) @ Wo = ctx @ Wo + bv @ Wo).

v2 design (vs v1 f32r baseline at ~322us):
  - All matmul operands bf16 (host-cast): FWL weight loads, half DMA,
    half DVE copy cost. PSUM accumulation stays f32, output f32.
  - Scores per head pair computed as two concurrent row-tiled K=64
    matmuls (tile_position=(64h,0)) -- halves scores PE time.
  - reciprocal_approx_fast (custom DVE, ~51 ULP) replaces the 3.3us
    bit-exact reciprocal in the softmax-denominator normalization.
  - One interleaved schedule: attention(pair0) starts as soon as the
    first K-chunk projections land (xt DMA'd qc-major); V projection and
    pair-1 K/Q projections and the output projection are spread into the
    ACT(exp)-bound attention windows so the PE never idles.
  - PSUM budget (8 banks): scores tag 2 bufs x 2 banks; ctx tag 3 bufs
    x 1 bank; aux (late proj + outproj) 1 buf x 1 bank.

Algebraic simplifications (exact, from v1):
  - bk dropped: softmax over k is invariant to the per-q constant Q.bk.
  - softmax without max subtraction (scores bounded, exp safe in f32).
  - bq folded into Q^T as a per-partition bias.
  - row normalization deferred past the P@V matmul (scale ctx instead
    of probs); row sums obtained free via an appended ones-column in V.
"""

import sys

sys.path.insert(0, "/opt/trn_rl_repo")

import numpy as np

B, S, D, H, HD = 2, 2048, 1024, 16, 64
HPC = 4  # heads per core
NCORES = 8
DC = D // 128  # 8 contraction chunks
ST = S // 128  # 16 s-tiles
QCW = 512  # q chunk width
QC = S // QCW  # 4 q chunks
KT = S // 128  # 16 k tiles

_CACHE = {}
APPROX_RECIP = True
DIRECT_OUT_DMA = False


def _build(repeat=1):
    import concourse.bass as bass  # noqa: F401
    import concourse.mybir as mybir
    import concourse.tile as tile
    from concourse import bacc

    F32 = mybir.dt.float32
    BF16 = mybir.dt.bfloat16
    AF = mybir.ActivationFunctionType

    nc = bacc.Bacc("TRN2", target_bir_lowering=False, debug=False)

    xt_d = nc.dram_tensor("xt", [D, S], BF16, kind="ExternalInput")
    wq_d = nc.dram_tensor("wq", [D, HPC * HD], BF16, kind="ExternalInput")
    wk_d = nc.dram_tensor("wk", [D, HPC * HD], BF16, kind="ExternalInput")
    wv_d = nc.dram_tensor("wv", [D, HPC * HD], BF16, kind="ExternalInput")
    wo_d = nc.dram_tensor("wo", [HPC * HD, D], BF16, kind="ExternalInput")
    bq_d = nc.dram_tensor("bq2", [128, 2], F32, kind="ExternalInput")
    out_d = nc.dram_tensor("out_p", [S, D], F32, kind="ExternalOutput")

    with tile.TileContext(nc) as tc:
        with (
            tc.tile_pool(name="wp", bufs=1) as wp,
            tc.tile_pool(name="xp", bufs=1) as xp,
            tc.tile_pool(name="qk", bufs=1) as qk,
            tc.tile_pool(name="vp", bufs=1) as vp,
            tc.tile_pool(name="ep", bufs=3) as ep,
            tc.tile_pool(name="cp", bufs=1) as cp,
            tc.tile_pool(name="mp", bufs=2) as mp,
            tc.tile_pool(name="op", bufs=2) as op,
            tc.tile_pool(name="pp", bufs=2, space="PSUM") as pp,
        ):
            # ---- one sync-queue DMA stream, ordered by first consumption:
            # wk/wq gate the first projections; xt is qc-major so pair-0
            # attention starts as soon as the first K-chunks land.
            wk_t = wp.tile([128, DC, HPC * HD], BF16, tag="wk")
            nc.sync.dma_start(wk_t[:], wk_d.rearrange("(c p) n -> p c n", p=128))
            xt_t = xp.tile([128, DC, S], BF16, tag="xt")
            xt_r = xt_d.rearrange("(c p) s -> p c s", p=128)

            def dma_xt(qc, half=None):
                qs = slice(qc * QCW, (qc + 1) * QCW)
                cs = slice(0, DC) if half is None else slice(half * DC // 2, (half + 1) * DC // 2)
                nc.sync.dma_start(xt_t[:, cs, qs], xt_r[:, cs, qs])

            dma_xt(0, 0)
            dma_xt(0, 1)
            wq_t = wp.tile([128, DC, HPC * HD], BF16, tag="wq")
            nc.sync.dma_start(wq_t[:], wq_d.rearrange("(c p) n -> p c n", p=128))
            bq_t = wp.tile([128, 2], F32, tag="bq")
            nc.sync.dma_start(bq_t[:], bq_d[:])
            wv_t = wp.tile([128, DC, HPC * HD], BF16, tag="wv")
            nc.sync.dma_start(wv_t[:], wv_d.rearrange("(c p) n -> p c n", p=128))
            dma_xt(1)
            wo_t = wp.tile([128, 2, D], BF16, tag="wo")
            nc.sync.dma_start(wo_t[:], wo_d.rearrange("(c p) n -> p c n", p=128))
            dma_xt(2)
            dma_xt(3)



            import contextlib
            if repeat > 1:
                _engs = [mybir.EngineType.PE, mybir.EngineType.Activation,
                         mybir.EngineType.DVE, mybir.EngineType.SP,
                         mybir.EngineType.Pool]
                rep_ctx = tc.For_i(0, repeat, hint_engines=_engs, staggered_reset=True)
            else:
                rep_ctx = contextlib.nullcontext()
            with rep_ctx:
                # ---- persistent SBUF tiles
                # v1 head block layout: [64 ones | 64 v-dims] so the PV matmul
                # replicates the softmax denominator onto PSUM partitions 0-63
                # (base-0 for the custom-DVE reciprocal) and ctx onto 64-127.
                v1_t = vp.tile([128, ST, HPC * 128], BF16, tag="v1")
                qt_tiles = [qk.tile([128, S], BF16, tag=f"qt{p}", name=f"qt{p}") for p in range(2)]
                kt_tiles = [qk.tile([128, S], BF16, tag=f"kt{p}", name=f"kt{p}") for p in range(2)]
                ctxt_tiles = [cp.tile([128, S], BF16, tag=f"ct{p}", name=f"ct{p}") for p in range(2)]

                def v_proj(st, tag="big"):
                    vps = pp.tile([128, HPC * HD], F32, tag=tag, name="vps",
                                  bufs=1 if tag == "aux" else 2)
                    for c in range(DC):
                        nc.tensor.matmul(
                            vps[:],
                            xt_t[:, c, st * 128:(st + 1) * 128],
                            wv_t[:, c, :],
                            start=(c == 0),
                            stop=(c == DC - 1),
                        )
                    with nc.allow_low_precision(reason="bf16 matmul operands"):
                        nc.vector.tensor_copy(
                            v1_t[:, st, :].rearrange("p (h c) -> p h c", c=128)[:, :, 64:128],
                            vps[:].rearrange("p (h c) -> p h c", c=64),
                        )

                def kt_proj(pair, qc, tag="big"):
                    qs = slice(qc * QCW, (qc + 1) * QCW)
                    kps = pp.tile([128, QCW], F32, tag=tag, name="kps",
                                  bufs=1 if tag == "aux" else 2)
                    for c in range(DC):
                        nc.tensor.matmul(
                            kps[:],
                            wk_t[:, c, pair * 128:(pair + 1) * 128],
                            xt_t[:, c, qs],
                            start=(c == 0),
                            stop=(c == DC - 1),
                        )
                    with nc.allow_low_precision(reason="bf16 matmul operands"):
                        nc.vector.tensor_copy(kt_tiles[pair][:, qs], kps[:])

                def qt_proj(pair, qc, tag="big"):
                    qs = slice(qc * QCW, (qc + 1) * QCW)
                    qps = pp.tile([128, QCW], F32, tag=tag, name="qps",
                                  bufs=1 if tag == "aux" else 2)
                    for c in range(DC):
                        nc.tensor.matmul(
                            qps[:],
                            wq_t[:, c, pair * 128:(pair + 1) * 128],
                            xt_t[:, c, qs],
                            start=(c == 0),
                            stop=(c == DC - 1),
                        )
                    with nc.allow_low_precision(reason="bf16 score operands"):
                        nc.vector.tensor_scalar_add(
                            qt_tiles[pair][:, qs], qps[:], bq_t[:, pair:pair + 1]
                        )

                # ---- software-pipelined attention: PV trails scores by 8
                # r-slots via staged expt tiles, so the PE never waits on the
                # ACT exp stream and the ACT engine is fed continuously.
                SEQ = [(p, q) for p in range(2) for q in range(QC)]
                expt_store = {}
                ctx_store = {}

                def sc_emit(w, r):
                    pair, qc = SEQ[w]
                    qs = slice(qc * QCW, (qc + 1) * QCW)
                    sreg = pp.tile([128, 2 * QCW], F32, tag="big")
                    expt = ep.tile([128, 2 * QCW], BF16, tag="exp", bufs=18)
                    for h in range(2):
                        nc.tensor.matmul(
                            sreg[:, h * QCW:(h + 1) * QCW],
                            kt_tiles[pair][64 * h:64 * (h + 1), r * 128:(r + 1) * 128],
                            qt_tiles[pair][64 * h:64 * (h + 1), qs],
                            start=True,
                            stop=True,
                            tile_position=(64 * h, 0),
                        )
                    nc.scalar.activation(expt[:], sreg[:], AF.Exp, scale=0.125)
                    expt_store[(w, r)] = expt

                def pv_emit(w, r):
                    pair, qc = SEQ[w]
                    if r == 0:
                        ctx_store[w] = [pp.tile([128, QCW], F32, tag="ctx", name=f"ctx{_h}", bufs=3) for _h in range(2)]
                    ctx_ps = ctx_store[w]
                    expt = expt_store.pop((w, r))
                    for h in range(2):
                        hh = 2 * pair + h
                        nc.tensor.matmul(
                            ctx_ps[h][:],
                            v1_t[:, r, 128 * hh:128 * hh + 128],
                            expt[:, h * QCW:(h + 1) * QCW],
                            start=(r == 0),
                            stop=(r == KT - 1),
                        )

                def norm_emit(w, norm_chunks=1):
                    pair, qc = SEQ[w]
                    qs = slice(qc * QCW, (qc + 1) * QCW)
                    ctx_ps = ctx_store.pop(w)
                    cw = QCW // norm_chunks
                    for cc in range(norm_chunks):
                        pcs = slice(cc * cw, (cc + 1) * cw)
                        for h in range(2):
                            bct = mp.tile([64, cw], F32, tag="bc")
                            if APPROX_RECIP:
                                nc.vector.reciprocal_approx_fast(bct[:], ctx_ps[h][0:64, pcs])
                            else:
                                nc.vector.reciprocal(bct[:], ctx_ps[h][0:64, pcs])
                            with nc.allow_low_precision(reason="bf16 matmul operands"):
                                nc.vector.tensor_mul(
                                    ctxt_tiles[pair][64 * h:64 * (h + 1), qs][:, pcs],
                                    ctx_ps[h][64:128, pcs],
                                    bct[:],
                                )

                def outproj(qc, subs=(0, 1, 2, 3), tail=False, psum_tag="aux"):
                    # tail=True: copies go to the scalar engine, which is idle
                    # once the exp stream has drained (keeps the in-order PE
                    # queue from stalling on DVE copy backpressure).
                    for sub in subs:
                        q0 = qc * QCW + sub * 128
                        for d2 in range(2):
                            ops = pp.tile([128, 512], F32, tag=psum_tag,
                                          bufs=1 if psum_tag == "aux" else 2, name="ops")
                            for pair in range(2):
                                nc.tensor.matmul(
                                    ops[:],
                                    ctxt_tiles[pair][:, q0:q0 + 128],
                                    wo_t[:, pair, d2 * 512:(d2 + 1) * 512],
                                    start=(pair == 0),
                                    stop=(pair == 1),
                                )
                            osb = op.tile([128, 512], F32, tag="osb")
                            if tail:
                                nc.scalar.copy(osb[:], ops[:])
                            else:
                                nc.vector.tensor_copy(osb[:], ops[:])
                            nc.sync.dma_start(out_d[q0:q0 + 128, d2 * 512:(d2 + 1) * 512], osb[:])

                # ================= schedule =================
                # Window w runs scores/exp for SEQ[w] while PV for the trailing
                # half of SEQ[w-1] and the leading half of SEQ[w] accumulate
                # from staged expt tiles. Projections/outproj fill PE slack.
                # head: run the first K/Q projections with interleaved
                # c-chunks so both trail the xt DMA tightly.
                kps0 = pp.tile([128, QCW], F32, tag="big", bufs=2, name="kps0")
                qps0 = pp.tile([128, QCW], F32, tag="big", bufs=2, name="qps0")
                for c in range(DC):
                    nc.tensor.matmul(kps0[:], wk_t[:, c, 0:128], xt_t[:, c, 0:QCW],
                                     start=(c == 0), stop=(c == DC - 1))
                    nc.tensor.matmul(qps0[:], wq_t[:, c, 0:128], xt_t[:, c, 0:QCW],
                                     start=(c == 0), stop=(c == DC - 1))
                with nc.allow_low_precision(reason="bf16 matmul operands"):
                    nc.vector.tensor_copy(kt_tiles[0][:, 0:QCW], kps0[:])
                    nc.vector.tensor_scalar_add(
                        qt_tiles[0][:, 0:QCW], qps0[:], bq_t[:, 0:1])
                nc.vector.memset(
                    v1_t[:].rearrange("p s (h c) -> p s h c", c=128)[:, :, :, 0:64],
                    1.0,
                )

                def hooks(w, r):
                    if w == 0:
                        if r == 2:
                            qt_proj(0, 1, tag="aux")
                        elif r in (4, 8, 12):
                            kt_proj(0, r // 4)
                        if r >= 8:
                            v_proj(r - 8)
                    elif w == 1:
                        if r < 8:
                            v_proj(r + 8)
                        if r == 2:
                            qt_proj(0, 2, tag="aux")
                    elif w == 2:
                        if r == 2:
                            qt_proj(0, 3, tag="aux")
                        elif r == 6:
                            kt_proj(1, 0, tag="aux")
                        elif r == 11:
                            kt_proj(1, 1, tag="aux")
                    elif w == 3:
                        if r == 2:
                            qt_proj(1, 0, tag="aux")
                        elif r == 6:
                            kt_proj(1, 2, tag="aux")
                        elif r == 11:
                            kt_proj(1, 3, tag="aux")
                    elif w == 4:
                        if r == 2:
                            qt_proj(1, 1, tag="aux")
                    elif w == 5:
                        if r == 2:
                            qt_proj(1, 2, tag="aux")
                        elif r == 9:
                            outproj(0, subs=(0, 1))
                        elif r == 12:
                            outproj(0, subs=(2, 3))
                    elif w == 6:
                        if r == 2:
                            qt_proj(1, 3, tag="aux")
                        elif r == 9:
                            outproj(1, subs=(0, 1))
                        elif r == 12:
                            outproj(1, subs=(2, 3))
                    elif w == 7:
                        if r in (9, 11, 13, 15):
                            outproj(2, subs=((r - 9) // 2,), tail=True)

                NW = len(SEQ)
                for w in range(NW):
                    for r in range(KT):
                        hooks(w, r)
                        sc_emit(w, r)
                        if r < 8:
                            if w > 0:
                                pv_emit(w - 1, r + 8)
                                if r == 7:
                                    norm_emit(w - 1)
                        else:
                            pv_emit(w, r - 8)
                # tail: drain the last window's PV, norm in chunks, project out
                for r in range(8, KT):
                    pv_emit(NW - 1, r)
                norm_emit(NW - 1, norm_chunks=4)
                outproj(QC - 1, tail=True, psum_tag="big")

    nc.compile()
    return nc


def _get_nc(repeat=1):
    key = (repeat, 3, APPROX_RECIP, DIRECT_OUT_DMA)
    if key not in _CACHE:
        _CACHE[key] = _build(repeat)
    return _CACHE[key]


def _make_in_maps(query_input, Wq, bq, Wk, Wv, Wo):
    from ml_dtypes import bfloat16

    x = np.asarray(query_input, dtype=np.float32)
    in_maps = []
    for core in range(NCORES):
        b, g = divmod(core, NCORES // B)
        cs = slice(g * HPC * HD, (g + 1) * HPC * HD)
        in_maps.append({
            "xt": np.ascontiguousarray(x[b].T).astype(bfloat16),
            "wq": np.ascontiguousarray(Wq[:, cs]).astype(bfloat16),
            "wk": np.ascontiguousarray(Wk[:, cs]).astype(bfloat16),
            "wv": np.ascontiguousarray(Wv[:, cs]).astype(bfloat16),
            "wo": np.ascontiguousarray(Wo[cs, :]).astype(bfloat16),
            "bq2": np.ascontiguousarray(bq[cs].reshape(2, 128).T),
        })
    return in_maps


def kernel(query_input, Wq, bq, Wk, bk, Wv, bv, Wo, bo):
    from concourse.bass_utils import run_bass_kernel_spmd

    Wq = np.asarray(Wq, np.float32)
    Wk = np.asarray(Wk, np.float32)
    Wv = np.asarray(Wv, np.float32)
    Wo = np.asarray(Wo, np.float32)
    bq = np.asarray(bq, np.float32)
    bv = np.asarray(bv, np.float32)
    bo = np.asarray(bo, np.float32)

    nc = _get_nc()
    in_maps = _make_in_maps(query_input, Wq, bq, Wk, Wv, Wo)
    res = run_bass_kernel_spmd(nc, in_maps, core_ids=list(range(NCORES)))

    gpc = NCORES // B  # groups per batch
    out = np.zeros((B, S, D), np.float32)
    for core in range(NCORES):
        b = core // gpc
        out[b] += res.results[core]["out_p"]
    # bv correction (exact) + bo, applied once on the full output
    out += (bv @ Wo + bo)[None, None, :]
    return out


# revision 58
# speedup vs baseline: 1.1670x; 1.0040x over previous
"""Self-contained 8-core Trainium2 Bass kernel for MultiHeadAttention.

Problem: B=2, S=2048, D=1024, H=16 heads (hd=64), f32, self-attention
(no mask), eval mode (dropout = identity).

Sharding: data-parallel over B (2) x tensor-parallel over heads (4 groups
of 4 heads) = 8 cores. Each core computes, for its batch b and its 4
heads: Q/K/V projections (column-sliced), attention, and a partial
output projection (row-sliced Wo). Host sums the 4 partials per batch
and adds the (bv @ Wo + bo) correction (bv never enters the kernel:
ctx rows sum probs to 1, so (ctx+bv) @ Wo = ctx @ Wo + bv @ Wo).

v2 design (vs v1 f32r baseline at ~322us):
  - All matmul operands bf16 (host-cast): FWL weight loads, half DMA,
    half DVE copy cost. PSUM accumulation stays f32, output f32.
  - Scores per head pair computed as two concurrent row-tiled K=64
    matmuls (tile_position=(64h,0)) -- halves scores PE time.
  - reciprocal_approx_fast (custom DVE, ~51 ULP) replaces the 3.3us
    bit-exact reciprocal in the softmax-denominator normalization.
  - One interleaved schedule: attention(pair0) starts as soon as the
    first K-chunk projections land (xt DMA'd qc-major); V projection and
    pair-1 K/Q projections and the output projection are spread into the
    ACT(exp)-bound attention windows so the PE never idles.
  - PSUM budget (8 banks): scores tag 2 bufs x 2 banks; ctx tag 3 bufs
    x 1 bank; aux (late proj + outproj) 1 buf x 1 bank.

Algebraic simplifications (exact, from v1):
  - bk dropped: softmax over k is invariant to the per-q constant Q.bk.
  - softmax without max subtraction (scores bounded, exp safe in f32).
  - bq folded into Q^T as a per-partition bias.
  - row normalization deferred past the P@V matmul (scale ctx instead
    of probs); row sums obtained free via an appended ones-column in V.
"""

import sys

sys.path.insert(0, "/opt/trn_rl_repo")

import numpy as np

B, S, D, H, HD = 2, 2048, 1024, 16, 64
HPC = 4  # heads per core
NCORES = 8
DC = D // 128  # 8 contraction chunks
ST = S // 128  # 16 s-tiles
QCW = 512  # q chunk width
QC = S // QCW  # 4 q chunks
KT = S // 128  # 16 k tiles

_CACHE = {}
APPROX_RECIP = True
DIRECT_OUT_DMA = False


def _build(repeat=1):
    import concourse.bass as bass  # noqa: F401
    import concourse.mybir as mybir
    import concourse.tile as tile
    from concourse import bacc

    F32 = mybir.dt.float32
    BF16 = mybir.dt.bfloat16
    AF = mybir.ActivationFunctionType

    nc = bacc.Bacc("TRN2", target_bir_lowering=False, debug=False)

    xt_d = nc.dram_tensor("xt", [D, S], BF16, kind="ExternalInput")
    wq_d = nc.dram_tensor("wq", [D, HPC * HD], BF16, kind="ExternalInput")
    wk_d = nc.dram_tensor("wk", [D, HPC * HD], BF16, kind="ExternalInput")
    wv_d = nc.dram_tensor("wv", [D, HPC * HD], BF16, kind="ExternalInput")
    wo_d = nc.dram_tensor("wo", [HPC * HD, D], BF16, kind="ExternalInput")
    bq_d = nc.dram_tensor("bq2", [128, 2], F32, kind="ExternalInput")
    out_d = nc.dram_tensor("out_p", [S, D], F32, kind="ExternalOutput")

    with tile.TileContext(nc) as tc:
        with (
            tc.tile_pool(name="wp", bufs=1) as wp,
            tc.tile_pool(name="xp", bufs=1) as xp,
            tc.tile_pool(name="qk", bufs=1) as qk,
            tc.tile_pool(name="vp", bufs=1) as vp,
            tc.tile_pool(name="ep", bufs=3) as ep,
            tc.tile_pool(name="cp", bufs=1) as cp,
            tc.tile_pool(name="mp", bufs=2) as mp,
            tc.tile_pool(name="op", bufs=2) as op,
            tc.tile_pool(name="pp", bufs=2, space="PSUM") as pp,
        ):
            # ---- one sync-queue DMA stream, ordered by first consumption:
            # wk/wq gate the first projections; xt is qc-major so pair-0
            # attention starts as soon as the first K-chunks land.
            wk_t = wp.tile([128, DC, HPC * HD], BF16, tag="wk")
            nc.sync.dma_start(wk_t[:], wk_d.rearrange("(c p) n -> p c n", p=128))
            xt_t = xp.tile([128, QC, DC, QCW], BF16, tag="xt")
            xt_r = xt_d.rearrange("(c p) (q w) -> p q c w", p=128, w=QCW)

            def dma_xt(qc, half=None):
                cs = slice(0, DC) if half is None else slice(half * DC // 2, (half + 1) * DC // 2)
                nc.sync.dma_start(xt_t[:, qc, cs, :], xt_r[:, qc, cs, :])

            dma_xt(0, 0)
            dma_xt(0, 1)
            wq_t = wp.tile([128, DC, HPC * HD], BF16, tag="wq")
            nc.sync.dma_start(wq_t[:], wq_d.rearrange("(c p) n -> p c n", p=128))
            bq_t = wp.tile([128, 2], F32, tag="bq")
            nc.sync.dma_start(bq_t[:], bq_d[:])
            wv_t = wp.tile([128, DC, HPC * HD], BF16, tag="wv")
            nc.sync.dma_start(wv_t[:], wv_d.rearrange("(c p) n -> p c n", p=128))
            dma_xt(1)
            wo_t = wp.tile([128, 2, D], BF16, tag="wo")
            nc.sync.dma_start(wo_t[:], wo_d.rearrange("(c p) n -> p c n", p=128))
            dma_xt(2)
            dma_xt(3)



            import contextlib
            if repeat > 1:
                _engs = [mybir.EngineType.PE, mybir.EngineType.Activation,
                         mybir.EngineType.DVE, mybir.EngineType.SP,
                         mybir.EngineType.Pool]
                rep_ctx = tc.For_i(0, repeat, hint_engines=_engs, staggered_reset=True)
            else:
                rep_ctx = contextlib.nullcontext()
            with rep_ctx:
                # ---- persistent SBUF tiles
                # v1 head block layout: [64 ones | 64 v-dims] so the PV matmul
                # replicates the softmax denominator onto PSUM partitions 0-63
                # (base-0 for the custom-DVE reciprocal) and ctx onto 64-127.
                v1_t = vp.tile([128, ST, HPC * 128], BF16, tag="v1")
                qt_tiles = [qk.tile([128, S], BF16, tag=f"qt{p}", name=f"qt{p}") for p in range(2)]
                kt_tiles = [qk.tile([128, S], BF16, tag=f"kt{p}", name=f"kt{p}") for p in range(2)]
                ctxt_tiles = [cp.tile([128, S], BF16, tag=f"ct{p}", name=f"ct{p}") for p in range(2)]

                def v_proj(st, tag="big"):
                    vps = pp.tile([128, HPC * HD], F32, tag=tag, name="vps",
                                  bufs=1 if tag == "aux" else 2)
                    for c in range(DC):
                        nc.tensor.matmul(
                            vps[:],
                            xt_t[:, st // 4, c, (st % 4) * 128:(st % 4) * 128 + 128],
                            wv_t[:, c, :],
                            start=(c == 0),
                            stop=(c == DC - 1),
                        )
                    with nc.allow_low_precision(reason="bf16 matmul operands"):
                        nc.vector.tensor_copy(
                            v1_t[:, st, :].rearrange("p (h c) -> p h c", c=128)[:, :, 64:128],
                            vps[:].rearrange("p (h c) -> p h c", c=64),
                        )

                def kt_proj(pair, qc, tag="big"):
                    qs = slice(qc * QCW, (qc + 1) * QCW)
                    kps = pp.tile([128, QCW], F32, tag=tag, name="kps",
                                  bufs=1 if tag == "aux" else 2)
                    for c in range(DC):
                        nc.tensor.matmul(
                            kps[:],
                            wk_t[:, c, pair * 128:(pair + 1) * 128],
                            xt_t[:, qc, c, :],
                            start=(c == 0),
                            stop=(c == DC - 1),
                        )
                    with nc.allow_low_precision(reason="bf16 matmul operands"):
                        nc.vector.tensor_copy(kt_tiles[pair][:, qs], kps[:])

                def qt_proj(pair, qc, tag="big"):
                    qs = slice(qc * QCW, (qc + 1) * QCW)
                    qps = pp.tile([128, QCW], F32, tag=tag, name="qps",
                                  bufs=1 if tag == "aux" else 2)
                    for c in range(DC):
                        nc.tensor.matmul(
                            qps[:],
                            wq_t[:, c, pair * 128:(pair + 1) * 128],
                            xt_t[:, qc, c, :],
                            start=(c == 0),
                            stop=(c == DC - 1),
                        )
                    with nc.allow_low_precision(reason="bf16 score operands"):
                        nc.vector.tensor_scalar_add(
                            qt_tiles[pair][:, qs], qps[:], bq_t[:, pair:pair + 1]
                        )

                # ---- software-pipelined attention: PV trails scores by 8
                # r-slots via staged expt tiles, so the PE never waits on the
                # ACT exp stream and the ACT engine is fed continuously.
                SEQ = [(p, q) for p in range(2) for q in range(QC)]
                expt_store = {}
                ctx_store = {}

                def sc_emit(w, r):
                    pair, qc = SEQ[w]
                    qs = slice(qc * QCW, (qc + 1) * QCW)
                    sreg = pp.tile([128, 2 * QCW], F32, tag="big")
                    expt = ep.tile([128, 2 * QCW], BF16, tag="exp", bufs=18)
                    for h in range(2):
                        nc.tensor.matmul(
                            sreg[:, h * QCW:(h + 1) * QCW],
                            kt_tiles[pair][64 * h:64 * (h + 1), r * 128:(r + 1) * 128],
                            qt_tiles[pair][64 * h:64 * (h + 1), qs],
                            start=True,
                            stop=True,
                            tile_position=(64 * h, 0),
                        )
                    nc.scalar.activation(expt[:], sreg[:], AF.Exp, scale=0.125)
                    expt_store[(w, r)] = expt

                def pv_emit(w, r):
                    pair, qc = SEQ[w]
                    if r == 0:
                        ctx_store[w] = [pp.tile([128, QCW], F32, tag="ctx", name=f"ctx{_h}", bufs=3) for _h in range(2)]
                    ctx_ps = ctx_store[w]
                    expt = expt_store.pop((w, r))
                    for h in range(2):
                        hh = 2 * pair + h
                        nc.tensor.matmul(
                            ctx_ps[h][:],
                            v1_t[:, r, 128 * hh:128 * hh + 128],
                            expt[:, h * QCW:(h + 1) * QCW],
                            start=(r == 0),
                            stop=(r == KT - 1),
                        )

                def norm_emit(w, norm_chunks=1):
                    pair, qc = SEQ[w]
                    qs = slice(qc * QCW, (qc + 1) * QCW)
                    ctx_ps = ctx_store.pop(w)
                    cw = QCW // norm_chunks
                    for cc in range(norm_chunks):
                        pcs = slice(cc * cw, (cc + 1) * cw)
                        for h in range(2):
                            bct = mp.tile([64, cw], F32, tag="bc")
                            if APPROX_RECIP:
                                nc.vector.reciprocal_approx_fast(bct[:], ctx_ps[h][0:64, pcs])
                            else:
                                nc.vector.reciprocal(bct[:], ctx_ps[h][0:64, pcs])
                            with nc.allow_low_precision(reason="bf16 matmul operands"):
                                nc.vector.tensor_mul(
                                    ctxt_tiles[pair][64 * h:64 * (h + 1), qs][:, pcs],
                                    ctx_ps[h][64:128, pcs],
                                    bct[:],
                                )

                def outproj(qc, subs=(0, 1, 2, 3), tail=False, psum_tag="aux"):
                    # tail=True: copies go to the scalar engine, which is idle
                    # once the exp stream has drained (keeps the in-order PE
                    # queue from stalling on DVE copy backpressure).
                    for sub in subs:
                        q0 = qc * QCW + sub * 128
                        for d2 in range(2):
                            ops = pp.tile([128, 512], F32, tag=psum_tag,
                                          bufs=1 if psum_tag == "aux" else 2, name="ops")
                            for pair in range(2):
                                nc.tensor.matmul(
                                    ops[:],
                                    ctxt_tiles[pair][:, q0:q0 + 128],
                                    wo_t[:, pair, d2 * 512:(d2 + 1) * 512],
                                    start=(pair == 0),
                                    stop=(pair == 1),
                                )
                            osb = op.tile([128, 512], F32, tag="osb")
                            if tail:
                                nc.scalar.copy(osb[:], ops[:])
                            else:
                                nc.vector.tensor_copy(osb[:], ops[:])
                            nc.sync.dma_start(out_d[q0:q0 + 128, d2 * 512:(d2 + 1) * 512], osb[:])

                # ================= schedule =================
                # Window w runs scores/exp for SEQ[w] while PV for the trailing
                # half of SEQ[w-1] and the leading half of SEQ[w] accumulate
                # from staged expt tiles. Projections/outproj fill PE slack.
                # head: run the first K/Q projections with interleaved
                # c-chunks so both trail the xt DMA tightly.
                kps0 = pp.tile([128, QCW], F32, tag="big", bufs=2, name="kps0")
                qps0 = pp.tile([128, QCW], F32, tag="big", bufs=2, name="qps0")
                for c in range(DC):
                    nc.tensor.matmul(kps0[:], wk_t[:, c, 0:128], xt_t[:, 0, c, :],
                                     start=(c == 0), stop=(c == DC - 1))
                    nc.tensor.matmul(qps0[:], wq_t[:, c, 0:128], xt_t[:, 0, c, :],
                                     start=(c == 0), stop=(c == DC - 1))
                with nc.allow_low_precision(reason="bf16 matmul operands"):
                    nc.vector.tensor_copy(kt_tiles[0][:, 0:QCW], kps0[:])
                    nc.vector.tensor_scalar_add(
                        qt_tiles[0][:, 0:QCW], qps0[:], bq_t[:, 0:1])
                nc.vector.memset(
                    v1_t[:].rearrange("p s (h c) -> p s h c", c=128)[:, :, :, 0:64],
                    1.0,
                )

                def hooks(w, r):
                    if w == 0:
                        if r == 2:
                            qt_proj(0, 1, tag="aux")
                        elif r in (4, 8, 12):
                            kt_proj(0, r // 4)
                        if r >= 8:
                            v_proj(r - 8)
                    elif w == 1:
                        if r < 8:
                            v_proj(r + 8)
                        if r == 2:
                            qt_proj(0, 2, tag="aux")
                    elif w == 2:
                        if r == 2:
                            qt_proj(0, 3, tag="aux")
                        elif r == 6:
                            kt_proj(1, 0, tag="aux")
                        elif r == 11:
                            kt_proj(1, 1, tag="aux")
                    elif w == 3:
                        if r == 2:
                            qt_proj(1, 0, tag="aux")
                        elif r == 6:
                            kt_proj(1, 2, tag="aux")
                        elif r == 11:
                            kt_proj(1, 3, tag="aux")
                    elif w == 4:
                        if r == 2:
                            qt_proj(1, 1, tag="aux")
                    elif w == 5:
                        if r == 2:
                            qt_proj(1, 2, tag="aux")
                        elif r == 9:
                            outproj(0, subs=(0, 1))
                        elif r == 12:
                            outproj(0, subs=(2, 3))
                    elif w == 6:
                        if r == 2:
                            qt_proj(1, 3, tag="aux")
                        elif r == 9:
                            outproj(1, subs=(0, 1))
                        elif r == 12:
                            outproj(1, subs=(2, 3))
                    elif w == 7:
                        if r in (9, 11, 13, 15):
                            outproj(2, subs=((r - 9) // 2,), tail=True)

                NW = len(SEQ)
                for w in range(NW):
                    for r in range(KT):
                        hooks(w, r)
                        sc_emit(w, r)
                        if r < 8:
                            if w > 0:
                                pv_emit(w - 1, r + 8)
                                if r == 7:
                                    norm_emit(w - 1)
                        else:
                            pv_emit(w, r - 8)
                # tail: drain the last window's PV, norm in chunks, project out
                for r in range(8, KT):
                    pv_emit(NW - 1, r)
                norm_emit(NW - 1, norm_chunks=4)
                outproj(QC - 1, tail=True, psum_tag="big")

    nc.compile()
    return nc


def _get_nc(repeat=1):
    key = (repeat, 3, APPROX_RECIP, DIRECT_OUT_DMA)
    if key not in _CACHE:
        _CACHE[key] = _build(repeat)
    return _CACHE[key]


def _make_in_maps(query_input, Wq, bq, Wk, Wv, Wo):
    from ml_dtypes import bfloat16

    x = np.asarray(query_input, dtype=np.float32)
    in_maps = []
    for core in range(NCORES):
        b, g = divmod(core, NCORES // B)
        cs = slice(g * HPC * HD, (g + 1) * HPC * HD)
        in_maps.append({
            "xt": np.ascontiguousarray(x[b].T).astype(bfloat16),
            "wq": np.ascontiguousarray(Wq[:, cs]).astype(bfloat16),
            "wk": np.ascontiguousarray(Wk[:, cs]).astype(bfloat16),
            "wv": np.ascontiguousarray(Wv[:, cs]).astype(bfloat16),
            "wo": np.ascontiguousarray(Wo[cs, :]).astype(bfloat16),
            "bq2": np.ascontiguousarray(bq[cs].reshape(2, 128).T),
        })
    return in_maps


def kernel(query_input, Wq, bq, Wk, bk, Wv, bv, Wo, bo):
    from concourse.bass_utils import run_bass_kernel_spmd

    Wq = np.asarray(Wq, np.float32)
    Wk = np.asarray(Wk, np.float32)
    Wv = np.asarray(Wv, np.float32)
    Wo = np.asarray(Wo, np.float32)
    bq = np.asarray(bq, np.float32)
    bv = np.asarray(bv, np.float32)
    bo = np.asarray(bo, np.float32)

    nc = _get_nc()
    in_maps = _make_in_maps(query_input, Wq, bq, Wk, Wv, Wo)
    res = run_bass_kernel_spmd(nc, in_maps, core_ids=list(range(NCORES)))

    gpc = NCORES // B  # groups per batch
    out = np.zeros((B, S, D), np.float32)
    for core in range(NCORES):
        b = core // gpc
        out[b] += res.results[core]["out_p"]
    # bv correction (exact) + bo, applied once on the full output
    out += (bv @ Wo + bo)[None, None, :]
    return out
